# revision 1
# baseline (speedup 1.0000x reference)
"""BasisConv GNN message passing on 8 TRN2 NeuronCores.

Strategy: sort edges by destination node, split into 8 shards at node
boundaries (each core owns a contiguous dst-node range -> collision-free
output, no all-reduce). Pack each shard into 128-edge tiles containing only
whole nodes (<=32 nodes/tile, dummy edges padded with out-of-range edge_attr
so their basis weights are exactly 0).

Per tile on-device:
  featT  = PE transpose of gathered x_j rows (4 tiles per transpose)
  Y      = featT.T @ Wflat           (PE, [128e, 16k*32o], one matmul)
  zz     = Y * b[e,k]                (DVE, one joint-AP multiply)
  outseg = sum_k S.T @ zz_k          (PE, 16 PSUM-accumulating matmuls:
                                      fuses k-contraction AND segment-sum)
  scatter outseg rows to dst nodes   (batched indirect DMA, unique rows)
"""

import os
import sys

for _p in ("/opt/trn_rl_repo", "/opt/pypackages"):
    if _p not in sys.path:
        sys.path.insert(0, _p)

import time

import numpy as np

import concourse.bacc as bacc
import concourse.bass as bass
import concourse.mybir as mybir
import concourse.tile as tile
from concourse import bass_utils

N_NODES = 50000
F = 32          # feature dim (in == out)
NB = 4          # basis terms per dimension
K = NB * NB     # 16 mixture terms
P = 128         # edges per tile
SEG = 32        # max segments (nodes) per tile
CH = 16         # tiles per chunk (one gather/scatter DMA per chunk)
GRP = 4         # tiles per PE-transpose / PSUM column group
NCORES = 8
DX = 2.0 / (NB - 1)          # hat basis spacing
CENTERS = np.linspace(-1.0, 1.0, NB, dtype=np.float32)
DUMMY_ATTR = 99.0            # basis value is exactly 0 out there
LAST_RESULTS = None          # BassKernelResults of the most recent run
LAST_TIMES = None            # wall times of repeat executions


def _pack_core(dst, src, attr, n0, n1, e0, e1):
    """Pack one core's (dst-sorted) edge range into whole-node 128-edge tiles.

    Returns per-tile slot arrays plus the node id of every (tile, seg) pair.
    Node ids are local (node - n0); nodes with >128 edges are split into
    pseudo-nodes that get spare rows appended after the range rows.
    """
    n_range = n1 - n0
    counts = np.bincount(dst[e0:e1] - n0, minlength=n_range)
    tiles = []          # list of (list of (local_node_or_spare_row, start_e, cnt))
    cur = []
    used = 0
    spares = []         # (true_local_node, spare_index)
    e = e0
    for ln in range(n_range):
        cnt = int(counts[ln])
        if cnt == 0:
            continue
        parts = []
        while cnt > P:
            parts.append(P)
            cnt -= P
        parts.append(cnt)
        for pi, pcnt in enumerate(parts):
            if pi == 0:
                row = ln
            else:
                row = n_range + len(spares)
                spares.append((ln, len(spares)))
            if used + pcnt > P or len(cur) >= SEG:
                tiles.append(cur)
                cur = []
                used = 0
            cur.append((row, e, pcnt))
            used += pcnt
            e += pcnt
    if cur:
        tiles.append(cur)
    return tiles, spares, n_range


def _build_device_arrays(tiles_list, spares_list, ranges, srcs, attrs, bounds_e):
    """Build the [128, T]-layout device input arrays for every core."""
    T = max(len(t) for t in tiles_list)
    T = ((T + CH - 1) // CH) * CH
    n_spare = max((len(s) for s in spares_list), default=0)
    RMAX = max(ranges) + n_spare
    ROWS = RMAX + 1               # last row is the trash row
    trash = ROWS - 1

    per_core = []
    for c in range(NCORES):
        tiles = tiles_list[c]
        src_il = np.zeros((P, T), np.int32)
        attr_il = np.full((P, T, 2), DUMMY_ATTR, np.float32)
        seg_il = np.zeros((P, T), np.float32)
        nid_il = np.full((P, T // GRP), -1, np.int64)   # host placement map
        for t, nodes in enumerate(tiles):
            p = 0
            g, j = divmod(t, GRP)
            for q, (row, e_start, cnt) in enumerate(nodes):
                sl = slice(p, p + cnt)
                src_il[sl, t] = srcs[c][e_start:e_start + cnt]
                attr_il[sl, t, :] = attrs[c][e_start:e_start + cnt]
                seg_il[sl, t] = q
                nid_il[32 * j + q, g] = row
                p += cnt
        per_core.append({
            "src_il": src_il,
            "attr_il": np.ascontiguousarray(attr_il.reshape(P, T * 2)),
            "seg_il": seg_il,
            "nid_il": nid_il,
        })
    return per_core, T, ROWS


def _build_nc(T, ROWS, debug_dump=False):
    nc = bacc.Bacc("TRN2", target_bir_lowering=False, debug=False,
                   enable_asserts=False, num_devices=NCORES)
    f32, i32 = mybir.dt.float32, mybir.dt.int32
    dbg = {}
    if debug_dump:
        dbg["feat"] = nc.dram_tensor("dbg_feat", [P, CH * F], f32, kind="ExternalOutput")
        dbg["bmat"] = nc.dram_tensor("dbg_bmat", [P, CH * K], f32, kind="ExternalOutput")
        dbg["smat"] = nc.dram_tensor("dbg_smat", [P, CH * SEG], f32, kind="ExternalOutput")
        dbg["zz"] = nc.dram_tensor("dbg_zz", [P, K * F], f32, kind="ExternalOutput")
        dbg["stage"] = nc.dram_tensor("dbg_stage", [P, (CH // GRP) * F], f32, kind="ExternalOutput")

    xj_d = nc.dram_tensor("xj", [N_NODES, F], f32, kind="ExternalInput")
    src_d = nc.dram_tensor("src_il", [P, T], i32, kind="ExternalInput")
    attr_d = nc.dram_tensor("attr_il", [P, T * 2], f32, kind="ExternalInput")
    seg_d = nc.dram_tensor("seg_il", [P, T], f32, kind="ExternalInput")
    wf_d = nc.dram_tensor("wflat4", [P, K * F], f32, kind="ExternalInput")
    id_d = nc.dram_tensor("ident", [P, P], f32, kind="ExternalInput")
    cen_d = nc.dram_tensor("cen8", [P, 2 * NB], f32, kind="ExternalInput")
    io_d = nc.dram_tensor("io32", [P, SEG], f32, kind="ExternalInput")
    out_d = nc.dram_tensor("out", [P, (T // GRP) * F], f32, kind="ExternalOutput")

    NC = T // CH       # chunks
    NG = CH // GRP     # groups per chunk

    with tile.TileContext(nc) as tc:
        with (
            tc.tile_pool(name="const", bufs=1) as cpool,
            tc.tile_pool(name="io", bufs=2) as iopool,
            tc.tile_pool(name="work", bufs=2) as wpool,
            tc.tile_pool(name="zzp", bufs=6) as zzpool,
            tc.tile_pool(name="ftp", bufs=2, space="PSUM") as ftpool,
            tc.tile_pool(name="yp", bufs=4, space="PSUM") as ypool,
            tc.tile_pool(name="sp", bufs=2, space="PSUM") as spool,
        ):
            wf = cpool.tile([P, K * F], f32, tag="wf")
            ident = cpool.tile([P, P], f32, tag="ident")
            cen = cpool.tile([P, 2 * NB], f32, tag="cen")
            io32 = cpool.tile([P, SEG], f32, tag="io")
            nc.sync.dma_start(wf[:], wf_d[:, :])
            nc.sync.dma_start(ident[:], id_d[:, :])
            nc.sync.dma_start(cen[:], cen_d[:, :])
            nc.sync.dma_start(io32[:], io_d[:, :])

            for c in range(NC):
                ts = slice(c * CH, (c + 1) * CH)
                idx = iopool.tile([P, CH], i32, tag="idx")
                attr = iopool.tile([P, CH * 2], f32, tag="attr")
                seg = iopool.tile([P, CH], f32, tag="seg")
                nc.sync.dma_start(idx[:], src_d[:, ts])
                nc.sync.dma_start(attr[:], attr_d[:, c * CH * 2:(c + 1) * CH * 2])
                nc.sync.dma_start(seg[:], seg_d[:, ts])

                feat = wpool.tile([P, CH * F], f32, tag="feat")
                for tl in range(CH):
                    nc.gpsimd.indirect_dma_start(
                        out=feat[:, tl * F:(tl + 1) * F],
                        out_offset=None, in_=xj_d[:, :],
                        in_offset=bass.IndirectOffsetOnAxis(
                            ap=idx[:, tl:tl + 1], axis=0))

                # hat basis for the whole chunk: [P, CH, 2, NB]
                bxy = wpool.tile([P, CH * 2 * NB], f32, tag="bxy")
                bxy_v = bxy[:].rearrange("p (t d n) -> p t d n", t=CH, d=2)
                nc.vector.tensor_tensor(
                    out=bxy_v,
                    in0=attr[:].rearrange("p (t d) -> p t d", d=2)
                        .unsqueeze(3).to_broadcast([P, CH, 2, NB]),
                    in1=cen[:].rearrange("p (d n) -> p d n", d=2)
                        .unsqueeze(1).to_broadcast([P, CH, 2, NB]),
                    op=mybir.AluOpType.subtract)
                nc.scalar.activation(
                    out=bxy[:], in_=bxy[:],
                    func=mybir.ActivationFunctionType.Abs,
                    scale=1.0 / DX)
                nc.scalar.activation(
                    out=bxy[:], in_=bxy[:],
                    func=mybir.ActivationFunctionType.Relu,
                    bias=1.0, scale=-1.0)
                # outer product b[p,t,a,c] = bx[p,t,a] * by[p,t,c]
                bmat = wpool.tile([P, CH * K], f32, tag="bmat")
                nc.vector.tensor_tensor(
                    out=bmat[:].rearrange("p (t a c) -> p t a c", t=CH, a=NB),
                    in0=bxy_v[:, :, 0, :].unsqueeze(3).to_broadcast([P, CH, NB, NB]),
                    in1=bxy_v[:, :, 1, :].unsqueeze(2).to_broadcast([P, CH, NB, NB]),
                    op=mybir.AluOpType.mult)
                # segment one-hot S[p,t,q] = (seg[p,t] == q)
                smat = wpool.tile([P, CH * SEG], f32, tag="smat")
                nc.vector.tensor_tensor(
                    out=smat[:].rearrange("p (t q) -> p t q", t=CH),
                    in0=seg[:].unsqueeze(2).to_broadcast([P, CH, SEG]),
                    in1=io32[:].unsqueeze(1).to_broadcast([P, CH, SEG]),
                    op=mybir.AluOpType.is_equal)

                stage = wpool.tile([P, NG * F], f32, tag="stage")
                for g in range(NG):
                    ft_ps = ftpool.tile([P, P], f32, tag="ft")
                    nc.tensor.transpose(
                        out=ft_ps[:], in_=feat[:, g * P:(g + 1) * P],
                        identity=ident[:])
                    ft_sb = wpool.tile([P, P], f32, tag="ftsb")
                    nc.scalar.activation(
                        out=ft_sb[:], in_=ft_ps[:],
                        func=mybir.ActivationFunctionType.Copy)
                    seg_ps = spool.tile([P, F], f32, tag="segps")
                    y_list, zz_list = [], []
                    for j in range(GRP):
                        y_ps = ypool.tile([P, K * F], f32, tag="y")
                        nc.tensor.matmul(
                            out=y_ps[:],
                            lhsT=ft_sb[32 * j:32 * (j + 1), :],
                            rhs=wf[32 * j:32 * (j + 1), :],
                            start=True, stop=True,
                            skip_group_check=True,
                            tile_position=(32 * j, 0))
                        y_list.append(y_ps)
                    for j in range(GRP):
                        tl = g * GRP + j
                        zz = zzpool.tile([P, K * F], f32, tag="zz")
                        nc.vector.tensor_tensor(
                            out=zz[:].rearrange("p (k o) -> p k o", k=K),
                            in0=y_list[j][:].rearrange("p (k o) -> p k o", k=K),
                            in1=bmat[:, tl * K:(tl + 1) * K]
                                .unsqueeze(2).to_broadcast([P, K, F]),
                            op=mybir.AluOpType.mult)
                        zz_list.append(zz)
                        if debug_dump and c == 0 and tl == 0:
                            nc.sync.dma_start(dbg["zz"][:, :], zz[:])
                    for j in range(GRP):
                        tl = g * GRP + j
                        for k in range(K):
                            nc.tensor.matmul(
                                out=seg_ps[32 * j:32 * (j + 1), :],
                                lhsT=smat[:, tl * SEG:(tl + 1) * SEG],
                                rhs=zz_list[j][:, k * F:(k + 1) * F],
                                start=(k == 0), stop=(k == K - 1),
                                skip_group_check=True,
                                tile_position=(0, 32 * j))
                    nc.scalar.activation(
                        out=stage[:, g * F:(g + 1) * F], in_=seg_ps[:],
                        func=mybir.ActivationFunctionType.Copy)
                if debug_dump and c == 0:
                    nc.sync.dma_start(dbg["feat"][:, :], feat[:])
                    nc.sync.dma_start(dbg["bmat"][:, :], bmat[:])
                    nc.sync.dma_start(dbg["smat"][:, :], smat[:])
                    nc.sync.dma_start(dbg["stage"][:, :], stage[:])
                nc.sync.dma_start(
                    out_d[:, c * NG * F:(c + 1) * NG * F], stage[:])

    nc.compile()
    return nc


def kernel(x_i, x_j, edge_index, edge_attr, weight):
    x_j = np.ascontiguousarray(np.asarray(x_j, np.float32))
    ei = np.asarray(edge_index)
    dst = ei[0].astype(np.int64)
    src = ei[1].astype(np.int64)
    attr = np.asarray(edge_attr, np.float32)
    w = np.asarray(weight, np.float32)
    E = dst.shape[0]

    order = np.argsort(dst, kind="stable")
    dst_s = dst[order]
    src_s = src[order].astype(np.int32)
    attr_s = attr[order]

    counts = np.bincount(dst_s, minlength=N_NODES)
    cume = np.concatenate([[0], np.cumsum(counts)])
    node_bounds = [0]
    for c in range(1, NCORES):
        node_bounds.append(int(np.searchsorted(cume, E * c // NCORES)))
    node_bounds.append(N_NODES)

    tiles_list, spares_list, ranges, srcs, attrs, ebounds = [], [], [], [], [], []
    for c in range(NCORES):
        n0, n1 = node_bounds[c], node_bounds[c + 1]
        e0, e1 = int(cume[n0]), int(cume[n1])
        tiles, spares, n_range = _pack_core(dst_s, src_s, attr_s, n0, n1, e0, e1)
        tiles_list.append(tiles)
        spares_list.append(spares)
        ranges.append(n_range)
        srcs.append(src_s)
        attrs.append(attr_s)
        ebounds.append((e0, e1))

    per_core, T, ROWS = _build_device_arrays(
        tiles_list, spares_list, ranges, srcs, attrs, ebounds)

    wflat = w.transpose(2, 0, 1, 3).reshape(F, K * F)        # [32i, (a c o)]
    wflat4 = np.ascontiguousarray(np.tile(wflat, (4, 1)))    # [128, 512]
    ident = np.eye(P, dtype=np.float32)
    cen8 = np.tile(np.concatenate([CENTERS, CENTERS])[None, :], (P, 1))
    io32 = np.tile(np.arange(SEG, dtype=np.float32)[None, :], (P, 1))

    nc = _build_nc(T, ROWS)

    in_maps = []
    for c in range(NCORES):
        m = dict(per_core[c])
        in_maps.append({
            "xj": x_j,
            "src_il": m["src_il"],
            "attr_il": m["attr_il"],
            "seg_il": m["seg_il"],
            "nid_il": m["nid_il"],
            "wflat4": wflat4.astype(np.float32),
            "ident": ident,
            "cen8": cen8.astype(np.float32),
            "io32": io32.astype(np.float32),
        })

    res = bass_utils.run_bass_kernel_spmd(nc, in_maps, core_ids=list(range(NCORES)))
    global LAST_RESULTS, LAST_TIMES
    LAST_RESULTS = res
    if os.environ.get("BC_TIME_REPEATS"):
        times = []
        for _ in range(int(os.environ["BC_TIME_REPEATS"])):
            t0 = time.time()
            bass_utils.run_bass_kernel_spmd(nc, in_maps, core_ids=list(range(NCORES)))
            times.append(time.time() - t0)
        LAST_TIMES = times

    out = np.zeros((N_NODES, F), np.float32)
    for c in range(NCORES):
        r = res.results[c]["out"].reshape(P, T // GRP, F)
        n0 = node_bounds[c]
        n_range = ranges[c]
        nid = per_core[c]["nid_il"]                  # [P, T//GRP] local rows
        pp, gg = np.nonzero(nid >= 0)
        rows = nid[pp, gg]
        vals = r[pp, gg, :]
        acc = np.zeros((n_range + len(spares_list[c]) + 1, F), np.float32)
        acc[rows] = vals
        out[n0:n0 + n_range] = acc[:n_range]
        for true_ln, si in spares_list[c]:
            out[n0 + true_ln] += acc[n_range + si]
    return out

